# revision 28
# baseline (speedup 1.0000x reference)
# MoE layer (all-experts dense MLP + weighted combine) on 8 TRN2 NeuronCores.
#
# Reference, for every token b (B=65536 total):
#   h_e   = relu(x @ W1[e] + b1[e])          e = 0..7
#   y_e   = h_e @ W2[e] + b2[e]
#   out_b = sum_e weights[b, e] * y_e
#
# Strategy (data-parallel over B, expert params replicated, 8192 tok/core):
#   - hdim-major tiles of 1024 tokens; hidden dim on partitions:
#       L1:  z_c[h', b] = W1_c^T @ x^T          (per 128-row chunk c=(e,m))
#       h_c = relu(z_c + b1_c)                  (ACT/DVE, per-partition bias)
#       hs_e = h_e * w_bcast_e                  (DVE + GPSIMD split)
#       L2:  po += W2_c^T @ hs_c ; b2^T @ w^T   (expert combine free in PSUM)
#     evac copy (DVE), DMA out transposed; host un-transposes.
#   - PE runs the pure matmul stream (no bias matmuls): 64 N=512 MMs + b2
#     per tile ~= 14.2us, the bf16 floor; LDWEIGHTS fully hidden (dedup'd).
#   - elementwise budget (relu 16 + hmul 8 + evac per tile) is spread over
#     ACT + DVE + GPSIMD; w broadcast via per-expert DRAM-src DMAs that
#     round-robin queues; input DMAs issued 2 tiles ahead; L2 lags L1 by
#     L2_LAG experts so PE never waits on a just-issued relu/hmul.
import numpy as np
import ml_dtypes

import concourse.bass as bass
import concourse.mybir as mybir
import concourse.tile as tile
from concourse.bass_utils import run_bass_kernel_spmd


E, D_IN, D_HID, D_OUT, B = 8, 128, 256, 128, 65536
N_CORES = 8
B_SHARD = B // N_CORES  # 8192
NB = 1024               # tokens per tile
NCHUNK = D_HID // 128   # 2 hidden-dim chunks per expert
NCH = E * NCHUNK        # 16 chunks

BF16 = mybir.dt.bfloat16
F32 = mybir.dt.float32
RELU = mybir.ActivationFunctionType.Relu

# chunks whose relu runs on DVE instead of ACT (tuning knob)
DVE_RELU = {1, 3, 5, 7}
L2_LAG = 3              # experts between L1 issue and L2 consume

_nc_cache = {}


def dedup_ldw(nc):
    """Drop redundant PE weight loads.

    Tile emits an InstLdweights before every InstMatmult; consecutive
    matmuls over the two 512-token halves of a tile reuse the same
    stationary weights, so the second load is a hardware no-op (weights
    persist in the PE array until the next load). Deleting it frees the
    PE background weight-buffer slot, letting boundary LDWEIGHTS pull
    ahead; its semaphore waits/updates are carried onto the next PE
    instruction (legalize_waits splits any overflow afterwards).
    """
    for f in nc.m.functions:
        for b in f.blocks:
            il = b.instructions
            out = []
            last_key = None
            carry_w, carry_u = [], []
            for inst in il:
                if inst.engine != mybir.EngineType.PE:
                    out.append(inst)
                    continue
                if isinstance(inst, mybir.InstLdweights):
                    key = str(inst.ins[0])
                    if key == last_key:
                        si = inst.sync_info
                        if si is not None:
                            carry_w.extend(list(si.on_wait))
                            carry_u.extend(list(si.on_update))
                        continue
                    last_key = key
                elif not isinstance(
                    inst, (mybir.InstMatmult, mybir.InstEventSemaphore)
                ):
                    last_key = None
                if carry_w or carry_u:
                    si = inst.sync_info
                    w = (list(si.on_wait) if si else []) + carry_w
                    u = (list(si.on_update) if si else []) + carry_u
                    inst.sync_info = mybir.SyncInfo(on_wait=w, on_update=u)
                    carry_w, carry_u = [], []
                out.append(inst)
            il[:] = out
    return nc


def legalize_waits(nc):
    """Split multi-wait instructions into standalone EventSemaphore waits.

    The walrus build in this container enforces the hardware sync-slot
    budget strictly: a normal instruction holds at most 1 sem wait (+1
    update); an EventSemaphore instruction holds 2. Tile's scheduler
    attaches up to 3 waits per instruction (and ~11 on the kernel-tail
    drain), which codegen rejects with "Too many sync wait commands".
    Hoisting the excess waits into standalone EventSemaphore instructions
    immediately before the op (same engine queue, so they gate execution
    identically) makes the program legal without changing semantics.
    """
    for f in nc.m.functions:
        for b in f.blocks:
            il = b.instructions
            out = []
            changed = False
            for inst in il:
                si = inst.sync_info
                if si is not None:
                    waits = list(si.on_wait)
                    upds = list(si.on_update)
                    assert len(upds) <= 1, f"{inst.name}: {len(upds)} updates"
                    cap = 2 if isinstance(inst, mybir.InstEventSemaphore) else 1
                    if len(waits) > cap:
                        extra, keep = waits[:-cap], waits[-cap:]
                        k = 0
                        while extra:
                            chunk, extra = extra[:2], extra[2:]
                            ev = mybir.InstEventSemaphore(
                                name=f"{inst.name}-lw{k}", ins=[], outs=[]
                            )
                            ev.engine = inst.engine
                            ev.sync_info = mybir.SyncInfo(
                                on_wait=chunk, on_update=[]
                            )
                            out.append(ev)
                            k += 1
                        inst.sync_info = mybir.SyncInfo(
                            on_wait=keep, on_update=upds
                        )
                        changed = True
                out.append(inst)
            if changed:
                il[:] = out
    return nc


def _rep(ap_2d, n):
    """View a [128, F] AP as [128, n, F] with a step-0 middle dim."""
    return bass.AP(
        tensor=ap_2d.tensor,
        offset=ap_2d.offset,
        ap=[ap_2d.ap[0], [0, n], ap_2d.ap[1]],
    )


def build_nc(b_shard=B_SHARD, nb=NB):
    assert b_shard % nb == 0
    n_tiles = b_shard // nb
    nsub = nb // 512
    nc = bass.Bass(trn_type="TRN2")

    xt = nc.dram_tensor("xt", [D_IN, b_shard], BF16, kind="ExternalInput").ap()
    wt = nc.dram_tensor("wt", [E, b_shard], BF16, kind="ExternalInput").ap()
    # W1 laid out [i, (e, m), h']: chunk c=(e,m) is lhsT for z_c rows m*128..
    w1l = nc.dram_tensor("w1l", [D_IN, NCH, 128], BF16, kind="ExternalInput").ap()
    # b1 laid out [p, (e, m)] = b1[e, m*128 + p]
    b1l = nc.dram_tensor("b1l", [128, NCH], F32, kind="ExternalInput").ap()
    # W2 laid out [h', (e, k), o]: chunk (e, k) is lhsT contracting h rows k*128..
    w2l = nc.dram_tensor("w2l", [128, NCH, D_OUT], BF16, kind="ExternalInput").ap()
    b2 = nc.dram_tensor("b2", [E, D_OUT], BF16, kind="ExternalInput").ap()
    outT = nc.dram_tensor("outT", [D_OUT, b_shard], BF16, kind="ExternalOutput").ap()

    with tile.TileContext(nc) as tc:
        with (
            tc.tile_pool(name="consts", bufs=1) as consts,
            tc.tile_pool(name="xt_p", bufs=4) as xt_p,
            tc.tile_pool(name="wt_p", bufs=4) as wt_p,
            tc.tile_pool(name="wbc_p", bufs=3) as wbc_p,
            tc.tile_pool(name="h_p", bufs=5) as h_p,
            tc.tile_pool(name="hs_p", bufs=5) as hs_p,
            tc.tile_pool(name="ot_p", bufs=2) as ot_p,
            tc.tile_pool(name="z_ps", bufs=3, space="PSUM") as z_ps,
            tc.tile_pool(name="o_ps", bufs=1, space="PSUM") as o_ps,
        ):
            w1_sbs = []
            for q in range(4):
                w1_q = consts.tile([D_IN, 4, 128], BF16, name=f"w1_q{q}")
                w1_sbs.append(w1_q)
            w2_sbs = []
            for q in range(2):
                w2_q = consts.tile([128, 8, D_OUT], BF16, name=f"w2_q{q}")
                w2_sbs.append(w2_q)
            b1_dma = consts.tile([128, NCH], F32, tag="b1_dma")
            b1_sb = consts.tile([128, NCH], F32, tag="b1_act")
            b2_sb = consts.tile([E, D_OUT], BF16)

            def issue_consts():
                # W1 is DMA'd inside the prologue issue_inputs; the rest
                # rides the scalar-engine ring in parallel
                for q in range(2):
                    cs = slice(8 * q, 8 * (q + 1))
                    nc.scalar.dma_start(w2_sbs[q], w2l[:, cs, :])
                nc.scalar.dma_start(b1_dma, b1l)
                # b1 laundered through an ACT copy so per-tile relus depend
                # on the Activation sem instead of a DMA sem
                nc.scalar.copy(b1_sb, b1_dma)
                nc.scalar.dma_start(b2_sb, b2)

            def issue_inputs(t, prologue=False):
                b0 = t * nb
                sl = slice(b0, b0 + nb)
                xt_sb = xt_p.tile([D_IN, nb], BF16, name="xt_sb")
                wt_sb = wt_p.tile([E, nb], BF16, name="wt_sb")
                wbc = wbc_p.tile([128, E, nb], BF16, name="wbc")

                def bcast(eng, e, ne=1):
                    eng.dma_start(
                        wbc[:, e : e + ne, :],
                        wt[e : e + ne, sl].partition_broadcast(128),
                    )

                if prologue:
                    # sync-ring order matched to first-tile consumption:
                    # W1 piece 0 + xt (first L1), wt, early experts' wbc,
                    # later W1 pieces interleaved with later wbc
                    nc.sync.dma_start(w1_sbs[0], w1l[:, 0:4, :])
                    h0 = slice(b0, b0 + nb // 2)
                    h1 = slice(b0 + nb // 2, b0 + nb)
                    nc.sync.dma_start(xt_sb[:, : nb // 2], xt[:, h0])
                    nc.sync.dma_start(xt_sb[:, nb // 2 :], xt[:, h1])
                    nc.sync.dma_start(wt_sb, wt[:, sl])
                    bcast(nc.sync, 0, 2)
                    for q in range(1, 4):
                        cs = slice(4 * q, 4 * (q + 1))
                        nc.sync.dma_start(w1_sbs[q], w1l[:, cs, :])
                        bcast(nc.sync, 2 * q, 2)
                else:
                    nc.sync.dma_start(xt_sb, xt[:, sl])
                    nc.sync.dma_start(wt_sb, wt[:, sl])
                    bcast(nc.gpsimd, 0, 4)
                    bcast(nc.sync, 4, 4)
                return xt_sb, wt_sb, wbc

            inputs = {0: issue_inputs(0, prologue=True)}
            issue_consts()

            for t in range(n_tiles):
                b0 = t * nb
                sl = slice(b0, b0 + nb)
                xt_sb, wt_sb, wbc = inputs[t]

                po = o_ps.tile([D_OUT, nb], F32, name="po")
                hs = [None] * E

                def l2_mms(e, first=False, last=False):
                    for k in range(NCHUNK):
                        c = NCHUNK * e + k
                        for j in range(nsub):
                            jsl = slice(512 * j, 512 * (j + 1))
                            nc.tensor.matmul(
                                po[:, jsl],
                                lhsT=w2_sbs[c // 8][:, c % 8, :],
                                rhs=hs[e][:, k, jsl],
                                start=(first and k == 0),
                                stop=(last and k == NCHUNK - 1 and j == nsub - 1),
                            )

                for e in range(E):
                    zs = []
                    for m in range(NCHUNK):
                        z = z_ps.tile([128, nb], F32, tag="z", name=f"z{e}_{m}")
                        zs.append(z)
                    # L1: z_c = W1_c^T @ x^T
                    for m in range(NCHUNK):
                        c = NCHUNK * e + m
                        for j in range(nsub):
                            jsl = slice(512 * j, 512 * (j + 1))
                            nc.tensor.matmul(
                                zs[m][:, jsl],
                                lhsT=w1_sbs[c // 4][:, c % 4, :], rhs=xt_sb[:, jsl],
                                start=True, stop=True,
                            )
                    # L2 lags by L2_LAG experts so PE never waits on relu/hmul
                    if e >= L2_LAG:
                        l2_mms(e - L2_LAG, first=(e == L2_LAG))
                    if e == L2_LAG + 1:
                        # b2 term: independent of the multiplies, so keep it
                        # out of the hmul-gated tile tail
                        for j in range(nsub):
                            jsl = slice(512 * j, 512 * (j + 1))
                            nc.tensor.matmul(
                                po[:, jsl], lhsT=b2_sb, rhs=wt_sb[:, jsl],
                                start=False, stop=False,
                            )
                    # prefetch next tiles' inputs mid-tile
                    if t == 0 and e == 0:
                        inputs[1] = issue_inputs(1)
                    if e == 2 and t + 2 < n_tiles:
                        inputs[t + 2] = issue_inputs(t + 2)
                    # relu + bias -> bf16 h planes
                    h = h_p.tile([128, NCHUNK, nb], BF16, tag="h", name=f"h{e}")
                    for m in range(NCHUNK):
                        c = NCHUNK * e + m
                        if c in DVE_RELU:
                            nc.vector.tensor_scalar(
                                h[:, m, :], zs[m],
                                b1_sb[:, c : c + 1], 0.0,
                                mybir.AluOpType.add, mybir.AluOpType.max,
                            )
                        else:
                            nc.scalar.activation(
                                h[:, m, :], zs[m], RELU,
                                bias=b1_sb[:, c : c + 1], scale=1.0,
                            )
                    # hs = h * w_bcast in one batched op per expert
                    hs[e] = hs_p.tile([128, NCHUNK, nb], BF16, tag="hs", name=f"hs{e}")
                    nc.vector.tensor_mul(hs[e], h, _rep(wbc[:, e, :], NCHUNK))
                # trailing experts' L2; the last one closes the group
                for e in range(E - L2_LAG, E):
                    l2_mms(e, last=(e == E - 1))

                ot = ot_p.tile([D_OUT, nb], BF16, name="ot")
                nc.scalar.copy(ot, po)
                if t == n_tiles - 1:
                    for q in range(4):
                        eng = nc.sync if q % 2 == 0 else nc.scalar
                        qs = slice(b0 + 256 * q, b0 + 256 * (q + 1))
                        eng.dma_start(outT[:, qs], ot[:, 256 * q : 256 * (q + 1)])
                else:
                    nc.sync.dma_start(outT[:, sl], ot)
    dedup_ldw(nc)
    return legalize_waits(nc)


def prep_consts(W1, b1, W2, b2):
    bf = ml_dtypes.bfloat16
    # w1l[i, c, h'] = W1[e, i, m*128 + h'],  c = 2e + m
    w1l = np.ascontiguousarray(
        W1.transpose(1, 0, 2).reshape(D_IN, E, NCHUNK, 128).reshape(D_IN, NCH, 128)
    ).astype(bf)
    # b1l[p, c] = b1[e, m*128 + p]
    b1l = np.ascontiguousarray(
        b1.reshape(E, NCHUNK, 128).transpose(2, 0, 1).reshape(128, NCH)
    ).astype(np.float32)
    # w2l[h', (e, k), o] = W2[e, k*128 + h', o]
    w2l = np.ascontiguousarray(
        W2.reshape(E, NCHUNK, 128, D_OUT).transpose(2, 0, 1, 3).reshape(128, NCH, D_OUT)
    ).astype(bf)
    return {"w1l": w1l, "b1l": b1l, "w2l": w2l, "b2": b2.astype(bf)}


def prep_core(x_c, w_c, consts):
    bf = ml_dtypes.bfloat16
    xt = np.ascontiguousarray(x_c.T).astype(bf)
    wt = np.ascontiguousarray(w_c.T).astype(bf)
    return {"xt": xt, "wt": wt, **consts}


def _ntff_hook():
    """NTFF profiling hook via the axon PJRT .so (the antenv.axon_hooks
    glue module is absent in this image, so wire it up directly)."""
    from trn_agent_boot.trn_boot import _ntff_profile_via_ctypes

    return _ntff_profile_via_ctypes("/opt/axon/libaxon_pjrt.so")


def run_traced(nc, in_maps, n_cores, out_dir):
    import concourse.bass2jax as bass2jax

    hook = _ntff_hook()
    with hook(out_dir, list(range(n_cores))):
        results = bass2jax.run_bass_via_pjrt(nc, in_maps, n_cores=n_cores)
    return results


def run(inputs, trace=False, b_shard=B_SHARD, nb=NB):
    x = np.asarray(inputs["x"], dtype=np.float32)
    w = np.asarray(inputs["weights"], dtype=np.float32)
    consts = prep_consts(
        np.asarray(inputs["W1"], dtype=np.float32),
        np.asarray(inputs["b1"], dtype=np.float32),
        np.asarray(inputs["W2"], dtype=np.float32),
        np.asarray(inputs["b2"], dtype=np.float32),
    )
    n_cores = x.shape[0] // b_shard
    key = (b_shard, nb)
    if key not in _nc_cache:
        _nc_cache[key] = build_nc(b_shard, nb)
    nc = _nc_cache[key]
    in_maps = [
        prep_core(
            x[c * b_shard : (c + 1) * b_shard],
            w[c * b_shard : (c + 1) * b_shard],
            consts,
        )
        for c in range(n_cores)
    ]
    if trace:
        import tempfile

        out_dir = tempfile.mkdtemp(prefix="moe_ntff_")
        results = run_traced(nc, in_maps, n_cores, out_dir)

        class _Res:
            pass

        res = _Res()
        res.results = results
        res.exec_time_ns = None
        res.trace_dir = out_dir
    else:
        res = run_bass_kernel_spmd(
            nc, in_maps, core_ids=list(range(n_cores)), trace=False
        )
        res.trace_dir = None
    out = np.concatenate([np.ascontiguousarray(r["outT"].T) for r in res.results], axis=0)
    return out.astype(np.float32), res


def kernel(**inputs) -> np.ndarray:
    out, _ = run(inputs)
    return out


# revision 29
# speedup vs baseline: 1.0776x; 1.0776x over previous
# MoE layer (all-experts dense MLP + weighted combine) on 8 TRN2 NeuronCores.
#
# Reference, for every token b (B=65536 total):
#   h_e   = relu(x @ W1[e] + b1[e])          e = 0..7
#   y_e   = h_e @ W2[e] + b2[e]
#   out_b = sum_e weights[b, e] * y_e
#
# Strategy (data-parallel over B, expert params replicated, 8192 tok/core):
#   - hdim-major tiles of 1024 tokens; hidden dim on partitions:
#       L1:  z_c[h', b] = W1_c^T @ x^T          (per 128-row chunk c=(e,m))
#       h_c = relu(z_c + b1_c)                  (ACT/DVE, per-partition bias)
#       hs_e = h_e * w_bcast_e                  (DVE + GPSIMD split)
#       L2:  po += W2_c^T @ hs_c ; b2^T @ w^T   (expert combine free in PSUM)
#     evac copy (DVE), DMA out transposed; host un-transposes.
#   - PE runs the pure matmul stream (no bias matmuls): 64 N=512 MMs + b2
#     per tile ~= 14.2us, the bf16 floor; LDWEIGHTS fully hidden (dedup'd).
#   - elementwise budget (relu 16 + hmul 8 + evac per tile) is spread over
#     ACT + DVE + GPSIMD; w broadcast via per-expert DRAM-src DMAs that
#     round-robin queues; input DMAs issued 2 tiles ahead; L2 lags L1 by
#     L2_LAG experts so PE never waits on a just-issued relu/hmul.
import numpy as np
import ml_dtypes

import concourse.bass as bass
import concourse.mybir as mybir
import concourse.tile as tile
from concourse.bass_utils import run_bass_kernel_spmd


E, D_IN, D_HID, D_OUT, B = 8, 128, 256, 128, 65536
N_CORES = 8
B_SHARD = B // N_CORES  # 8192
NB = 1024               # tokens per tile
NCHUNK = D_HID // 128   # 2 hidden-dim chunks per expert
NCH = E * NCHUNK        # 16 chunks

BF16 = mybir.dt.bfloat16
F32 = mybir.dt.float32
RELU = mybir.ActivationFunctionType.Relu

# chunks whose relu runs on DVE instead of ACT (tuning knob)
DVE_RELU = {1, 3, 5, 7}
L2_LAG = 3              # experts between L1 issue and L2 consume

_nc_cache = {}


def dedup_ldw(nc):
    """Drop redundant PE weight loads.

    Tile emits an InstLdweights before every InstMatmult; consecutive
    matmuls over the two 512-token halves of a tile reuse the same
    stationary weights, so the second load is a hardware no-op (weights
    persist in the PE array until the next load). Deleting it frees the
    PE background weight-buffer slot, letting boundary LDWEIGHTS pull
    ahead; its semaphore waits/updates are carried onto the next PE
    instruction (legalize_waits splits any overflow afterwards).
    """
    for f in nc.m.functions:
        for b in f.blocks:
            il = b.instructions
            out = []
            last_key = None
            carry_w, carry_u = [], []
            for inst in il:
                if inst.engine != mybir.EngineType.PE:
                    out.append(inst)
                    continue
                if isinstance(inst, mybir.InstLdweights):
                    key = str(inst.ins[0])
                    if key == last_key:
                        si = inst.sync_info
                        if si is not None:
                            carry_w.extend(list(si.on_wait))
                            carry_u.extend(list(si.on_update))
                        continue
                    last_key = key
                elif not isinstance(
                    inst, (mybir.InstMatmult, mybir.InstEventSemaphore)
                ):
                    last_key = None
                if carry_w or carry_u:
                    si = inst.sync_info
                    w = (list(si.on_wait) if si else []) + carry_w
                    u = (list(si.on_update) if si else []) + carry_u
                    inst.sync_info = mybir.SyncInfo(on_wait=w, on_update=u)
                    carry_w, carry_u = [], []
                out.append(inst)
            il[:] = out
    return nc


def legalize_waits(nc):
    """Split multi-wait instructions into standalone EventSemaphore waits.

    The walrus build in this container enforces the hardware sync-slot
    budget strictly: a normal instruction holds at most 1 sem wait (+1
    update); an EventSemaphore instruction holds 2. Tile's scheduler
    attaches up to 3 waits per instruction (and ~11 on the kernel-tail
    drain), which codegen rejects with "Too many sync wait commands".
    Hoisting the excess waits into standalone EventSemaphore instructions
    immediately before the op (same engine queue, so they gate execution
    identically) makes the program legal without changing semantics.
    """
    for f in nc.m.functions:
        for b in f.blocks:
            il = b.instructions
            out = []
            changed = False
            for inst in il:
                si = inst.sync_info
                if si is not None:
                    waits = list(si.on_wait)
                    upds = list(si.on_update)
                    assert len(upds) <= 1, f"{inst.name}: {len(upds)} updates"
                    cap = 2 if isinstance(inst, mybir.InstEventSemaphore) else 1
                    if len(waits) > cap:
                        extra, keep = waits[:-cap], waits[-cap:]
                        k = 0
                        while extra:
                            chunk, extra = extra[:2], extra[2:]
                            ev = mybir.InstEventSemaphore(
                                name=f"{inst.name}-lw{k}", ins=[], outs=[]
                            )
                            ev.engine = inst.engine
                            ev.sync_info = mybir.SyncInfo(
                                on_wait=chunk, on_update=[]
                            )
                            out.append(ev)
                            k += 1
                        inst.sync_info = mybir.SyncInfo(
                            on_wait=keep, on_update=upds
                        )
                        changed = True
                out.append(inst)
            if changed:
                il[:] = out
    return nc


def _rep(ap_2d, n):
    """View a [128, F] AP as [128, n, F] with a step-0 middle dim."""
    return bass.AP(
        tensor=ap_2d.tensor,
        offset=ap_2d.offset,
        ap=[ap_2d.ap[0], [0, n], ap_2d.ap[1]],
    )


def build_nc(b_shard=B_SHARD, nb=NB):
    assert b_shard % nb == 0
    n_tiles = b_shard // nb
    nsub = nb // 512
    nc = bass.Bass(trn_type="TRN2")

    xt = nc.dram_tensor("xt", [D_IN, b_shard], BF16, kind="ExternalInput").ap()
    wt = nc.dram_tensor("wt", [E, b_shard], BF16, kind="ExternalInput").ap()
    # W1 laid out [i, (e, m), h']: chunk c=(e,m) is lhsT for z_c rows m*128..
    w1l = nc.dram_tensor("w1l", [D_IN, NCH, 128], BF16, kind="ExternalInput").ap()
    # b1 laid out [p, (e, m)] = b1[e, m*128 + p]
    b1l = nc.dram_tensor("b1l", [128, NCH], F32, kind="ExternalInput").ap()
    # W2 laid out [h', (e, k), o]: chunk (e, k) is lhsT contracting h rows k*128..
    w2l = nc.dram_tensor("w2l", [128, NCH, D_OUT], BF16, kind="ExternalInput").ap()
    b2 = nc.dram_tensor("b2", [E, D_OUT], BF16, kind="ExternalInput").ap()
    outT = nc.dram_tensor("outT", [D_OUT, b_shard], BF16, kind="ExternalOutput").ap()

    with tile.TileContext(nc) as tc:
        with (
            tc.tile_pool(name="consts", bufs=1) as consts,
            tc.tile_pool(name="xt_p", bufs=4) as xt_p,
            tc.tile_pool(name="wt_p", bufs=4) as wt_p,
            tc.tile_pool(name="wbc_p", bufs=3) as wbc_p,
            tc.tile_pool(name="h_p", bufs=5) as h_p,
            tc.tile_pool(name="hs_p", bufs=5) as hs_p,
            tc.tile_pool(name="ot_p", bufs=2) as ot_p,
            tc.tile_pool(name="z_ps", bufs=3, space="PSUM") as z_ps,
            tc.tile_pool(name="o_ps", bufs=1, space="PSUM") as o_ps,
        ):
            w1_sbs = []
            for q in range(4):
                w1_q = consts.tile([D_IN, 4, 128], BF16, name=f"w1_q{q}")
                w1_sbs.append(w1_q)
            w2_sbs = []
            for q in range(2):
                w2_q = consts.tile([128, 8, D_OUT], BF16, name=f"w2_q{q}")
                w2_sbs.append(w2_q)
            b1_dma = consts.tile([128, NCH], F32, tag="b1_dma")
            b1_sb = consts.tile([128, NCH], F32, tag="b1_act")
            b2_sb = consts.tile([E, D_OUT], BF16)

            def issue_consts():
                # W1 is DMA'd inside the prologue issue_inputs; the rest
                # rides the scalar-engine ring in parallel
                for q in range(2):
                    cs = slice(8 * q, 8 * (q + 1))
                    nc.scalar.dma_start(w2_sbs[q], w2l[:, cs, :])
                nc.scalar.dma_start(b1_dma, b1l)
                # b1 laundered through an ACT copy so per-tile relus depend
                # on the Activation sem instead of a DMA sem
                nc.scalar.copy(b1_sb, b1_dma)
                nc.scalar.dma_start(b2_sb, b2)

            def issue_inputs(t, prologue=False):
                b0 = t * nb
                sl = slice(b0, b0 + nb)
                xt_sb = xt_p.tile([D_IN, nb], BF16, name="xt_sb")
                wt_sb = wt_p.tile([E, nb], BF16, name="wt_sb")
                wbc = wbc_p.tile([128, E, nb], BF16, name="wbc")

                def bcast(eng, e, ne=1):
                    eng.dma_start(
                        wbc[:, e : e + ne, :],
                        wt[e : e + ne, sl].partition_broadcast(128),
                    )

                if prologue:
                    # sync-ring order matched to first-tile consumption:
                    # W1 piece 0 + xt (first L1), wt, early experts' wbc,
                    # later W1 pieces interleaved with later wbc
                    nc.sync.dma_start(w1_sbs[0], w1l[:, 0:4, :])
                    h0 = slice(b0, b0 + nb // 2)
                    h1 = slice(b0 + nb // 2, b0 + nb)
                    nc.sync.dma_start(xt_sb[:, : nb // 2], xt[:, h0])
                    nc.sync.dma_start(xt_sb[:, nb // 2 :], xt[:, h1])
                    nc.sync.dma_start(wt_sb, wt[:, sl])
                    bcast(nc.sync, 0, 2)
                    for q in range(1, 4):
                        cs = slice(4 * q, 4 * (q + 1))
                        nc.sync.dma_start(w1_sbs[q], w1l[:, cs, :])
                        bcast(nc.sync, 2 * q, 2)
                else:
                    nc.sync.dma_start(xt_sb, xt[:, sl])
                    nc.sync.dma_start(wt_sb, wt[:, sl])
                    bcast(nc.gpsimd, 0, 4)
                    bcast(nc.sync, 4, 4)
                return xt_sb, wt_sb, wbc

            inputs = {0: issue_inputs(0, prologue=True)}
            issue_consts()

            for t in range(n_tiles):
                b0 = t * nb
                sl = slice(b0, b0 + nb)
                xt_sb, wt_sb, wbc = inputs[t]

                po = o_ps.tile([D_OUT, nb], F32, name="po")
                hs = [None] * E

                def l2_mms(e, first=False):
                    for k in range(NCHUNK):
                        c = NCHUNK * e + k
                        for j in range(nsub):
                            jsl = slice(512 * j, 512 * (j + 1))
                            nc.tensor.matmul(
                                po[:, jsl],
                                lhsT=w2_sbs[c // 8][:, c % 8, :],
                                rhs=hs[e][:, k, jsl],
                                start=(first and k == 0),
                                stop=False,
                            )

                for e in range(E):
                    zs = []
                    for m in range(NCHUNK):
                        z = z_ps.tile([128, nb], F32, tag="z", name=f"z{e}_{m}")
                        zs.append(z)
                    # L1: z_c = W1_c^T @ x^T
                    for m in range(NCHUNK):
                        c = NCHUNK * e + m
                        for j in range(nsub):
                            jsl = slice(512 * j, 512 * (j + 1))
                            nc.tensor.matmul(
                                zs[m][:, jsl],
                                lhsT=w1_sbs[c // 4][:, c % 4, :], rhs=xt_sb[:, jsl],
                                start=True, stop=True,
                            )
                    # L2 lags by L2_LAG experts so PE never waits on relu/hmul
                    if e >= L2_LAG:
                        l2_mms(e - L2_LAG, first=(e == L2_LAG))
                    # prefetch next tiles' inputs mid-tile
                    if t == 0 and e == 0:
                        inputs[1] = issue_inputs(1)
                    if e == 2 and t + 2 < n_tiles:
                        inputs[t + 2] = issue_inputs(t + 2)
                    # relu + bias -> bf16 h planes
                    h = h_p.tile([128, NCHUNK, nb], BF16, tag="h", name=f"h{e}")
                    for m in range(NCHUNK):
                        c = NCHUNK * e + m
                        if c in DVE_RELU:
                            nc.vector.tensor_scalar(
                                h[:, m, :], zs[m],
                                b1_sb[:, c : c + 1], 0.0,
                                mybir.AluOpType.add, mybir.AluOpType.max,
                            )
                        else:
                            nc.scalar.activation(
                                h[:, m, :], zs[m], RELU,
                                bias=b1_sb[:, c : c + 1], scale=1.0,
                            )
                    # hs = h * w_bcast in one batched op per expert
                    hs[e] = hs_p.tile([128, NCHUNK, nb], BF16, tag="hs", name=f"hs{e}")
                    nc.vector.tensor_mul(hs[e], h, _rep(wbc[:, e, :], NCHUNK))
                # trailing experts' L2, then the b2 term closes the group
                for e in range(E - L2_LAG, E):
                    l2_mms(e)
                for j in range(nsub):
                    jsl = slice(512 * j, 512 * (j + 1))
                    nc.tensor.matmul(
                        po[:, jsl], lhsT=b2_sb, rhs=wt_sb[:, jsl],
                        start=False, stop=(j == nsub - 1),
                    )

                ot = ot_p.tile([D_OUT, nb], BF16, name="ot")
                nc.scalar.copy(ot, po)
                if t == n_tiles - 1:
                    for q in range(4):
                        eng = nc.sync if q % 2 == 0 else nc.scalar
                        qs = slice(b0 + 256 * q, b0 + 256 * (q + 1))
                        eng.dma_start(outT[:, qs], ot[:, 256 * q : 256 * (q + 1)])
                else:
                    nc.sync.dma_start(outT[:, sl], ot)
    dedup_ldw(nc)
    return legalize_waits(nc)


def prep_consts(W1, b1, W2, b2):
    bf = ml_dtypes.bfloat16
    # w1l[i, c, h'] = W1[e, i, m*128 + h'],  c = 2e + m
    w1l = np.ascontiguousarray(
        W1.transpose(1, 0, 2).reshape(D_IN, E, NCHUNK, 128).reshape(D_IN, NCH, 128)
    ).astype(bf)
    # b1l[p, c] = b1[e, m*128 + p]
    b1l = np.ascontiguousarray(
        b1.reshape(E, NCHUNK, 128).transpose(2, 0, 1).reshape(128, NCH)
    ).astype(np.float32)
    # w2l[h', (e, k), o] = W2[e, k*128 + h', o]
    w2l = np.ascontiguousarray(
        W2.reshape(E, NCHUNK, 128, D_OUT).transpose(2, 0, 1, 3).reshape(128, NCH, D_OUT)
    ).astype(bf)
    return {"w1l": w1l, "b1l": b1l, "w2l": w2l, "b2": b2.astype(bf)}


def prep_core(x_c, w_c, consts):
    bf = ml_dtypes.bfloat16
    xt = np.ascontiguousarray(x_c.T).astype(bf)
    wt = np.ascontiguousarray(w_c.T).astype(bf)
    return {"xt": xt, "wt": wt, **consts}


def _ntff_hook():
    """NTFF profiling hook via the axon PJRT .so (the antenv.axon_hooks
    glue module is absent in this image, so wire it up directly)."""
    from trn_agent_boot.trn_boot import _ntff_profile_via_ctypes

    return _ntff_profile_via_ctypes("/opt/axon/libaxon_pjrt.so")


def run_traced(nc, in_maps, n_cores, out_dir):
    import concourse.bass2jax as bass2jax

    hook = _ntff_hook()
    with hook(out_dir, list(range(n_cores))):
        results = bass2jax.run_bass_via_pjrt(nc, in_maps, n_cores=n_cores)
    return results


def run(inputs, trace=False, b_shard=B_SHARD, nb=NB):
    x = np.asarray(inputs["x"], dtype=np.float32)
    w = np.asarray(inputs["weights"], dtype=np.float32)
    consts = prep_consts(
        np.asarray(inputs["W1"], dtype=np.float32),
        np.asarray(inputs["b1"], dtype=np.float32),
        np.asarray(inputs["W2"], dtype=np.float32),
        np.asarray(inputs["b2"], dtype=np.float32),
    )
    n_cores = x.shape[0] // b_shard
    key = (b_shard, nb)
    if key not in _nc_cache:
        _nc_cache[key] = build_nc(b_shard, nb)
    nc = _nc_cache[key]
    in_maps = [
        prep_core(
            x[c * b_shard : (c + 1) * b_shard],
            w[c * b_shard : (c + 1) * b_shard],
            consts,
        )
        for c in range(n_cores)
    ]
    if trace:
        import tempfile

        out_dir = tempfile.mkdtemp(prefix="moe_ntff_")
        results = run_traced(nc, in_maps, n_cores, out_dir)

        class _Res:
            pass

        res = _Res()
        res.results = results
        res.exec_time_ns = None
        res.trace_dir = out_dir
    else:
        res = run_bass_kernel_spmd(
            nc, in_maps, core_ids=list(range(n_cores)), trace=False
        )
        res.trace_dir = None
    out = np.concatenate([np.ascontiguousarray(r["outT"].T) for r in res.results], axis=0)
    return out.astype(np.float32), res


def kernel(**inputs) -> np.ndarray:
    out, _ = run(inputs)
    return out
